# revision 18
# baseline (speedup 1.0000x reference)
"""Trainium2 Bass kernel for nn_DA_84825604096359.

Strip-pooling dual-direction attention + CBAM channel attention.

Math: the reference reduces to
    out[b,c,h,w] = x * (A'[b,c] + R1[b,c,h] + S''[b,c,w])
with A' = 1 + beta*ca, R1 = alpha*lam_h*w_h, S'' = alpha*lam_w*w_w + A'.

Sharding: batch item b -> core b (8 items, 8 cores), no communication.

Per-core schedule (tuned against the TimelineSim cost model):
  - x loads as fp16 (cast rides the SWDGE descriptors). C-tile E loads
    first; c-tile L second with a SMALL final chunk, so the global
    channel-attention scalar A' (which needs every channel's mean) is
    ready ~1us after the last byte lands.
  - PE computes all four SUM stats as PSUM-accumulated fp16 identity
    matmuls (cost is output-columns only; the contraction is free).
  - DVE computes row-max trees (fp16 2x); Pool computes leaf+L1 of the
    col-max trees; DVE finishes the tails.
  - 13-tap dilated depthwise conv = 13 PSUM-accumulated diag(weff)
    matmuls on PE against a zero-guarded fp16 s; ACT pre-builds the 52
    diag tiles during the load and folds BN+sigmoid into PSUM drains.
  - Final phase pipelines PE (M = R1 + S'' built with identr matmuls
    against broadcast f32r views), ACT (PSUM -> fp16 drain), DVE
    (o = x * m16, fp16 2x) and Pool (fp16->f32 cast-DMA out) per
    32-row group. E's groups ship first; all of L's heavy tree/conv
    work happens lazily during the out phase.
"""

import numpy as np

import concourse.bacc as bacc
import concourse.mybir as mybir
from concourse.bass_utils import run_bass_kernel_spmd
from concourse.masks import make_identity
from concourse.tile import TileContext

B, C, H, W = 8, 256, 128, 128
K = 7
DILS = (1, 2, 3)
HIDDEN = C // 16
EPS = 1e-5
P = 128
NCT = C // P

F32 = mybir.dt.float32
F32R = mybir.dt.float32r
F16 = mybir.dt.float16
Alu = mybir.AluOpType
Act = mybir.ActivationFunctionType
AxX = mybir.AxisListType.X

OFFSETS = sorted({d * (k - 3) for d in DILS for k in range(K)})  # 13 offsets
NOFF = len(OFFSETS)
GUARD = max(-OFFSETS[0], OFFSETS[-1])  # 9
LG = GUARD + H + GUARD

E, L = 0, 1                      # early / late c-tile roles
CHUNKS = {E: ((0, 16), (16, 16), (32, 32), (64, 32), (96, 32)),
          L: ((0, 32), (32, 32), (64, 32), (96, 16), (112, 16))}
GROUP = 32                       # h-rows per out-DMA group
FCH = 8                          # h-rows per M PSUM chunk
NSUB = 512 // W                  # rows per 512-col M matmul


def _fold_params(inputs):
    f = {k: np.asarray(v, dtype=np.float32) for k, v in inputs.items()}
    out = {}
    for tag, pfx in (("h", "hw"), ("w", "ww")):
        conv = f[f"{pfx}_conv"]            # (3, C, 1, K)
        g, b = f[f"{pfx}_bn_g"], f[f"{pfx}_bn_b"]
        m, v = f[f"{pfx}_bn_m"], f[f"{pfx}_bn_v"]
        p = g / np.sqrt(v + EPS)
        q = b - p * m
        weff = np.zeros((C, NOFF), np.float32)
        for i, d in enumerate(DILS):
            for k in range(K):
                weff[:, OFFSETS.index(d * (k - 3))] += conv[i, :, 0, k]
        out[f"weff_{tag}"] = weff * p[:, None]           # BN scale folded
        out[f"q_{tag}"] = q.reshape(C, 1)
        sq_w, sq_b = f[f"{pfx}_sq_w"], f[f"{pfx}_sq_b"]
        out[f"c0_{tag}"] = float(sq_w[0])
        out[f"c1_{tag}"] = float(sq_w[1]) / (W if tag == "h" else H)
        out[f"sqb_{tag}"] = float(sq_b[0])
    gp = f["gate_bn_g"] / np.sqrt(f["gate_bn_v"] + EPS)
    out["gate_a"] = (gp * f["gate_w"]).reshape(C, 1)
    out["gate_b"] = (f["gate_bn_b"] - gp * f["gate_bn_m"]).reshape(C, 1)
    mw, mb = f["mix_W"], f["mix_b"]
    out["u0"] = float(mw[0, 0] - mw[1, 0]) / H
    out["u1"] = float(mw[0, 1] - mw[1, 1]) / H
    out["u2"] = float(mb[0] - mb[1])
    out["fc1t"] = np.ascontiguousarray(f["ca_fc1"].T)    # (C, HIDDEN)
    out["fc2t"] = np.ascontiguousarray(f["ca_fc2"].T)    # (HIDDEN, C)
    out["alpha"] = float(f["alpha"])
    out["beta"] = float(f["beta"])
    return out


MARKS = []


def _build(pr, ablate=(), reps=1):
    MARKS.clear()

    def mark(label):
        # record the next instruction index for trace attribution
        nxt = nc.get_next_instruction_name()      # consumes one name
        MARKS.append((int(nxt.split("-")[1]), label))

    nc = bacc.Bacc("TRN2", target_bir_lowering=False, debug=False)

    x = nc.dram_tensor("x", [C, H, W], F32, kind="ExternalInput")
    weff_h = nc.dram_tensor("weff_h", [C, NOFF], F32, kind="ExternalInput")
    weff_w = nc.dram_tensor("weff_w", [C, NOFF], F32, kind="ExternalInput")
    q_h = nc.dram_tensor("q_h", [C, 1], F32, kind="ExternalInput")
    q_w = nc.dram_tensor("q_w", [C, 1], F32, kind="ExternalInput")
    gate_a = nc.dram_tensor("gate_a", [C, 1], F32, kind="ExternalInput")
    gate_b = nc.dram_tensor("gate_b", [C, 1], F32, kind="ExternalInput")
    fc1t = nc.dram_tensor("fc1t", [C, HIDDEN], F32, kind="ExternalInput")
    fc2t = nc.dram_tensor("fc2t", [HIDDEN, C], F32, kind="ExternalInput")
    out = nc.dram_tensor("out", [C, H, W], F32, kind="ExternalOutput")

    with TileContext(nc) as tc:
        with (
            tc.tile_pool(name="xpool", bufs=1) as xpool,
            tc.tile_pool(name="tree", bufs=1) as treep,
            tc.tile_pool(name="params", bufs=1) as params,
            tc.tile_pool(name="small", bufs=1) as small,
            tc.tile_pool(name="diags", bufs=1) as diagp,
            tc.tile_pool(name="junk", bufs=2) as junkp,
            tc.tile_pool(name="mpool", bufs=3) as mpool,
            tc.tile_pool(name="opool", bufs=3) as opool,
            tc.tile_pool(name="psum", bufs=1, space="PSUM") as psum,
            tc.tile_pool(name="psm", bufs=2, space="PSUM") as psm,
        ):
            def _rep_body():
                # ======== in-DMA triggers (Pool): E chunks then L ========
                ident16 = params.tile([P, P], F16, tag="ident16")
                make_identity(nc, ident16[:])
                xts = {}
                for ct in (E, L):
                    cs = slice(ct * P, (ct + 1) * P)
                    xt = xpool.tile([P, H, W], F16, tag=f"x{ct}",
                                    name=f"xt{ct}")
                    xts[ct] = xt
                    for h0, hn in CHUNKS[ct]:
                        nc.gpsimd.dma_start(xt[:, h0:h0 + hn, :],
                                            x[cs, h0:h0 + hn, :])

                # ======== param loads on SP HWDGE ========
                ptiles = {}
                for ct in (E, L):
                    cs = slice(ct * P, (ct + 1) * P)
                    for nm, dram in (("weff_h", weff_h), ("weff_w", weff_w),
                                     ("q_h", q_h), ("q_w", q_w),
                                     ("gate_a", gate_a), ("gate_b", gate_b),
                                     ("fc1t", fc1t)):
                        t = params.tile([P, dram.shape[1]], F32,
                                        tag=f"{nm}{ct}")
                        nc.sync.dma_start(t[:], dram[cs, :])
                        ptiles[(nm, ct)] = t
                fc2_t = params.tile([HIDDEN, C], F32, tag="fc2t")
                nc.sync.dma_start(fc2_t[:], fc2t[:])
                ones11 = params.tile([1, 1], F32, tag="ones11")
                nc.vector.memset(ones11[:], 1.0)
                identr = params.tile([P, P], F32R, tag="identr")
                nc.vector.tensor_copy(identr[:], ident16[:])

                # warm the ACT function tables off the critical path
                warm = small.tile([1, 1], F32, tag="warm")
                nc.scalar.activation(warm[:], ones11[:], Act.Sigmoid)
                nc.scalar.activation(warm[:], ones11[:], Act.Relu)

                # ======== guarded fp16 s tiles ========
                s16g = {}
                for tag in ("h", "w"):
                    for ct in (E, L):
                        t = small.tile([P, LG], F16, tag=f"s{tag}{ct}")
                        nc.vector.memset(t[:, 0:GUARD], 0.0)
                        nc.vector.memset(t[:, GUARD + H:LG], 0.0)
                        s16g[(tag, ct)] = t

                # ======== ACT: diag(weff) tiles during the load ========
                diag = {}

                def build_diags(ct, only=None):
                    for tag in ("w", "h"):
                        if only is not None and tag != only:
                            continue
                        wt = ptiles[(f"weff_{tag}", ct)]
                        for i in range(NOFF):
                            d = diagp.tile([P, P], F16, tag=f"dg{tag}{ct}_{i}")
                            nc.scalar.activation(d[:], ident16[:], Act.Copy,
                                                 scale=wt[:, i:i + 1])
                            diag[(tag, ct, i)] = d
                build_diags(E)

                # ======== PSUM regions ========
                ps_cs = psum.tile([P, 2, W], F32, tag="ps_cs")
                ps_rs = psum.tile([P, 2, H], F32, tag="ps_rs")
                ps_y = psum.tile([P, 4, H], F32, tag="ps_y")
                ps_ca = psum.tile([P, 2 * HIDDEN + 3], F32, tag="ps_ca")
                hid_ps = ps_ca[0:1, 0:2 * HIDDEN]
                hT_ps = ps_ca[0:HIDDEN, 2 * HIDDEN:2 * HIDDEN + 1]
                ca_ps = ps_ca[:, 2 * HIDDEN + 1:2 * HIDDEN + 3]
                YIDX = {("h", E): 0, ("w", E): 1, ("h", L): 2, ("w", L): 3}

                # ======== tree scratch ========
                trA = treep.tile([P, 4096], F16, tag="trA")
                trB = treep.tile([P, 2048], F16, tag="trB")
                trE_ = treep.tile([P, 1024], F16, tag="trE_")
                trF = treep.tile([P, 512], F16, tag="trF")
                trP = treep.tile([P, 4096], F16, tag="trP")
                trL2 = [treep.tile([P, 2048], F16, tag=f"trL2_{i}",
                                   name=f"trL2_{i}") for i in range(5)]

                def view(t, a, b):
                    return t[:, 0:a * b].rearrange("p (a b) -> p a b", b=b)

                def tree(dst, src, n_keep, n_red, op, red_h, eng=None,
                         bufs=None):
                    eng = eng or nc.vector
                    bufs = bufs or (trA, trB)
                    cur, n = src, n_red
                    pp = 0
                    while n > 2:
                        half = n // 2
                        buf = bufs[pp]
                        if red_h:
                            nxt = view(buf, half, n_keep)
                            eng.tensor_tensor(nxt[:], cur[:, 0:half, :],
                                              cur[:, half:n, :], op)
                        else:
                            nxt = view(buf, n_keep, half)
                            eng.tensor_tensor(nxt[:], cur[:, :, 0:half],
                                              cur[:, :, half:n], op)
                        cur, n, pp = nxt, half, 1 - pp
                    if red_h:
                        eng.tensor_tensor(dst, cur[:, 0, :], cur[:, 1, :], op)
                    else:
                        eng.tensor_tensor(dst, cur[:, :, 0:1].squeeze(2),
                                          cur[:, :, 1:2].squeeze(2), op)

                rmax = {ct: small.tile([P, H], F32, tag=f"rmax{ct}",
                                       name=f"rmax{ct}") for ct in (E, L)}
                cmax = {ct: small.tile([P, W], F32, tag=f"cmax{ct}",
                                       name=f"cmax{ct}") for ct in (E, L)}
                cm_t = small.tile([P, W], F32, tag="cm_t")

                # ======== engine helper emitters ========
                def pool_cmax_piece(ct, ci):
                    """Leaf + L1 of the col-max tree for one chunk.
                    (Pool cannot run TensorTensor(max) per the ISA, so
                    these run on DVE too.)"""
                    h0, hn = CHUNKS[ct][ci]
                    src = xts[ct][:, h0:h0 + hn, :]
                    l1 = view(trP, hn // 2, W)
                    nc.vector.tensor_tensor(l1[:], src[:, 0:hn // 2, :],
                                            src[:, hn // 2:hn, :], Alu.max)
                    l2 = view(trL2[ci % 5], hn // 4, W)
                    nc.vector.tensor_tensor(l2[:], l1[:, 0:hn // 4, :],
                                            l1[:, hn // 4:hn // 2, :],
                                            Alu.max)

                def dve_cmax_tail(ct, ci):
                    """DVE: finish the col-max tree for one chunk, combine."""
                    h0, hn = CHUNKS[ct][ci]
                    l2 = view(trL2[ci % 5], hn // 4, W)
                    dst = cmax[ct] if ci == 0 else cm_t
                    if hn // 4 >= 2:
                        tree(dst[:], l2, W, hn // 4, Alu.max, True,
                             bufs=(trE_, trF))
                    else:
                        nc.vector.tensor_copy(dst[:], l2[:, 0, :])
                    if ci > 0:
                        nc.vector.tensor_tensor(cmax[ct][:], cmax[ct][:],
                                                cm_t[:], Alu.max)

                def pe_colsum(ct):
                    """PE: one accumulation group over all chunks."""
                    reg = ps_cs[:, ct, :]
                    idx = 0
                    for h0, hn in CHUNKS[ct]:
                        for j in range(hn):
                            nc.tensor.matmul(
                                reg, lhsT=ident16[:],
                                rhs=xts[ct][:, h0 + j, :],
                                start=(idx == 0), stop=(idx == H - 1),
                                skip_group_check=True)
                            idx += 1

                def pe_rowsum_half(ct, h0):
                    reg = ps_rs[:, ct, h0:h0 + 64]
                    for j in range(W):
                        nc.tensor.matmul(
                            reg, lhsT=ident16[:],
                            rhs=xts[ct][:, h0:h0 + 64, j:j + 1].squeeze(2),
                            start=(j == 0), stop=(j == W - 1),
                            skip_group_check=True)

                def pe_conv(tag, ct):
                    reg = ps_y[:, YIDX[(tag, ct)], :]
                    sg = s16g[(tag, ct)]
                    for i, off in enumerate(OFFSETS):
                        nc.tensor.matmul(
                            reg, lhsT=diag[(tag, ct, i)][:],
                            rhs=sg[:, GUARD + off:GUARD + off + H],
                            start=(i == 0), stop=(i == NOFF - 1),
                            skip_group_check=True)

                def act_s_affine(tag, ct, psrc):
                    mid = s16g[(tag, ct)][:, GUARD:GUARD + H]
                    nc.scalar.activation(mid, psrc, Act.Copy,
                                         bias=pr[f"sqb_{tag}"],
                                         scale=pr[f"c1_{tag}"])

                def dve_s_max(tag, ct, pmax_t):
                    mid = s16g[(tag, ct)][:, GUARD:GUARD + H]
                    nc.vector.scalar_tensor_tensor(
                        mid, pmax_t[:], pr[f"c0_{tag}"], mid,
                        op0=Alu.mult, op1=Alu.add)

                def act_sigmoid_gate(tag, ct):
                    wd = small.tile([P, H], F16, tag=f"wd{tag}{ct}")
                    nc.scalar.activation(wd[:], ps_y[:, YIDX[(tag, ct)], :],
                                         Act.Sigmoid,
                                         bias=ptiles[(f"q_{tag}", ct)][:, 0:1])
                    junk = junkp.tile([P, H], F16, tag="junk")
                    gacc = small.tile([P, 1], F32, tag=f"g{tag}{ct}")
                    nc.scalar.activation(junk[:], wd[:], Act.Relu,
                                         bias=ptiles[("gate_b", ct)][:, 0:1],
                                         scale=ptiles[("gate_a", ct)][:, 0:1],
                                         accum_out=gacc[:])
                    return wd, gacc

                def dve_lam(ct, gh, gw, wh):
                    """lam from gates; returns (lamw_a, R1 f32r)."""
                    d = small.tile([P, 1], F32, tag=f"d{ct}")
                    nc.vector.tensor_scalar(d[:], gh[:], pr["u0"], pr["u2"],
                                            Alu.mult, Alu.add)
                    nc.vector.scalar_tensor_tensor(d[:], gw[:], pr["u1"],
                                                   d[:], op0=Alu.mult,
                                                   op1=Alu.add)
                    lamh = small.tile([P, 1], F32, tag=f"lamh{ct}")
                    nc.scalar.activation(lamh[:], d[:], Act.Sigmoid)
                    lamh_a = small.tile([P, 1], F32, tag=f"lamha{ct}")
                    nc.vector.tensor_scalar(lamh_a[:], lamh[:], pr["alpha"],
                                            None, Alu.mult)
                    lamw_a = small.tile([P, 1], F32, tag=f"lamwa{ct}")
                    nc.vector.tensor_scalar(lamw_a[:], lamh[:], -pr["alpha"],
                                            pr["alpha"], Alu.mult, Alu.add)
                    r1 = small.tile([P, H], F32R, tag=f"r1{ct}")
                    nc.vector.tensor_scalar(r1[:], wh[:], lamh_a[:, 0:1],
                                            None, Alu.mult)
                    return lamw_a, r1

                def dve_spp(ct, lamw_a, ww, ap):
                    """S'' = lamw_a*w_w + A' (f32r, broadcast add)."""
                    sp = small.tile([P, W], F32R, tag=f"sp{ct}")
                    nc.vector.scalar_tensor_tensor(
                        sp[:], ww[:], lamw_a[:, 0:1],
                        ap[:, 0:1].broadcast_to([P, W]),
                        op0=Alu.mult, op1=Alu.add)
                    return sp

                def group_rows(ct, g0, r1, sp16, ap):
                    """Shallow path: DVE builds M rows at 4x, multiplies,
                    ships. No PE/ACT involvement (lowest latency)."""
                    o16 = opool.tile([P, GROUP, W], F16, tag="o16")
                    m16 = mpool.tile([P, GROUP, W], F16, tag="m16g",
                                     name="m16g")
                    for j in range(GROUP):
                        nc.vector.tensor_scalar(
                            m16[:, j, :], sp16[:],
                            r1[:, g0 + j:g0 + j + 1].bitcast(F32),
                            ap[:, 0:1], Alu.add, Alu.add)
                    nc.vector.tensor_tensor(
                        o16[:], xts[ct][:, g0:g0 + GROUP, :], m16[:],
                        Alu.mult)
                    cs = slice(ct * P, (ct + 1) * P)
                    nc.gpsimd.dma_start(out[cs, g0:g0 + GROUP, :], o16[:])

                def group_out(ct, g0, r1, sp):
                    """PE M-build + ACT drain + DVE multiply + Pool out."""
                    o16 = opool.tile([P, GROUP, W], F16, tag="o16")
                    for c0 in range(g0, g0 + GROUP, FCH):
                        mps = psm.tile([P, FCH, W], F32, tag="mps")
                        for j in range(FCH // NSUB):
                            reg = mps[:, j * NSUB:(j + 1) * NSUB, :]
                            nc.tensor.matmul(
                                reg, lhsT=identr[:],
                                rhs=sp[:].unsqueeze(1)
                                    .broadcast_to([P, NSUB, W]),
                                start=True, stop=False,
                                skip_group_check=True)
                            nc.tensor.matmul(
                                reg, lhsT=identr[:],
                                rhs=r1[:, c0 + j * NSUB:c0 + (j + 1) * NSUB]
                                    .unsqueeze(2).broadcast_to([P, NSUB, W]),
                                start=False, stop=True,
                                skip_group_check=True)
                        m16 = mpool.tile([P, FCH, W], F16, tag="m16")
                        nc.scalar.copy(m16[:], mps[:])
                        nc.vector.tensor_tensor(
                            o16[:, c0 - g0:c0 - g0 + FCH, :],
                            xts[ct][:, c0:c0 + FCH, :], m16[:], Alu.mult)
                    cs = slice(ct * P, (ct + 1) * P)
                    nc.gpsimd.dma_start(out[cs, g0:g0 + GROUP, :], o16[:])

                # ================================================
                # phase 1: DVE does row-max trees; Pool does leaf+L1 of
                # BOTH col-max trees (DVE finishes tails)
                # ================================================
                mark("p1-start")
                NE, NL = len(CHUNKS[E]), len(CHUNKS[L])
                for ci in range(NE):
                    h0, hn = CHUNKS[E][ci]
                    tree(rmax[E][:, h0:h0 + hn], xts[E][:, h0:h0 + hn, :],
                         hn, W, Alu.max, False)          # DVE
                    pool_cmax_piece(E, ci)               # Pool
                for ci in range(NE):
                    dve_cmax_tail(E, ci)                 # DVE

                mark("pe_colsum_E")
                pe_colsum(E)                             # PE
                pe_rowsum_half(E, 0)                     # PE
                pe_rowsum_half(E, 64)                    # PE

                # E small chains (during L load); L diags fill ACT gaps
                mark("sw_E_chain")
                act_s_affine("w", E, ps_cs[:, E, :])
                build_diags(L, "w")
                dve_s_max("w", E, cmax[E])
                pe_conv("w", E)
                mark("sh_E_chain")
                act_s_affine("h", E, ps_rs[:, E, :])
                build_diags(L, "h")
                dve_s_max("h", E, rmax[E])
                mark("gate_w_E")
                wwE, gwE = act_sigmoid_gate("w", E)
                pe_conv("h", E)
                mark("gate_h_E")
                whE, ghE = act_sigmoid_gate("h", E)

                # L colsum: the A' gate — next on PE after E's work
                mark("pe_colsum_L")
                pe_colsum(L)

                # DVE: L row-max trees as chunks land; lam_E slotted in
                h0, hn = CHUNKS[L][0]
                tree(rmax[L][:, h0:h0 + hn], xts[L][:, h0:h0 + hn, :],
                     hn, W, Alu.max, False)
                mark("lam_E")
                lamw_aE, r1E = dve_lam(E, ghE, gwE, whE)
                for ci in range(1, NL):
                    h0, hn = CHUNKS[L][ci]
                    tree(rmax[L][:, h0:h0 + hn], xts[L][:, h0:h0 + hn, :],
                         hn, W, Alu.max, False)

                # Pool: lazy cmax_L pieces (gate nothing until ~45us)
                mark("cmaxL_p0")
                pool_cmax_piece(L, 0)
                pool_cmax_piece(L, 1)

                mark("CA_chain")
                # CA pooled vectors + MLP -> A'
                vmean, vmax = {}, {}
                for ct in (E, L):
                    gs = small.tile([P, 1], F32, tag=f"gs{ct}")
                    nc.vector.tensor_reduce(gs[:], ps_cs[:, ct, :],
                                            axis=AxX, op=Alu.add)
                    vm = small.tile([P, 1], F32, tag=f"vmean{ct}")
                    nc.vector.tensor_scalar(vm[:], gs[:], 1.0 / (H * W),
                                            None, Alu.mult)
                    vmean[ct] = vm
                    vx = small.tile([P, 1], F32, tag=f"vmax{ct}")
                    nc.vector.tensor_reduce(vx[:], rmax[ct][:], axis=AxX,
                                            op=Alu.max)
                    vmax[ct] = vx
                for ct in (E, L):
                    nc.tensor.matmul(hid_ps[:, 0:HIDDEN],
                                     lhsT=vmean[ct][:, 0:1],
                                     rhs=ptiles[("fc1t", ct)][:],
                                     start=(ct == E), stop=(ct == L),
                                     skip_group_check=True)
                for ct in (E, L):
                    nc.tensor.matmul(hid_ps[:, HIDDEN:2 * HIDDEN],
                                     lhsT=vmax[ct][:, 0:1],
                                     rhs=ptiles[("fc1t", ct)][:],
                                     start=(ct == E), stop=(ct == L),
                                     skip_group_check=True)
                hrelu = small.tile([1, 2 * HIDDEN], F32, tag="hrelu")
                nc.scalar.activation(hrelu[:], hid_ps[:], Act.Relu)
                hsum = small.tile([1, HIDDEN], F32, tag="hsum")
                nc.vector.tensor_tensor(hsum[:], hrelu[:, 0:HIDDEN],
                                        hrelu[:, HIDDEN:2 * HIDDEN], Alu.add)
                nc.tensor.transpose(hT_ps, hsum[:], ones11[:])
                hT = small.tile([HIDDEN, 1], F32, tag="hTs")
                nc.vector.tensor_copy(hT[:], hT_ps)
                aprime = {}
                for ct in (E, L):
                    cs = slice(ct * P, (ct + 1) * P)
                    nc.tensor.matmul(ca_ps[:, ct:ct + 1], lhsT=fc2_t[:, cs],
                                     rhs=hT[:], start=True, stop=True)
                    ca = small.tile([P, 1], F32, tag=f"cas{ct}")
                    nc.scalar.activation(ca[:], ca_ps[:, ct:ct + 1],
                                         Act.Sigmoid)
                    ap = small.tile([P, 1], F32, tag=f"ap{ct}")
                    nc.vector.tensor_scalar(ap[:], ca[:], pr["beta"], 1.0,
                                            Alu.mult, Alu.add)
                    aprime[ct] = ap
                mark("spE")
                spE = dve_spp(E, lamw_aE, wwE, aprime[E])

                # ================================================
                # phase 3: E groups out; L's heavy work is lazy
                # ================================================
                mark("G0")
                group_out(E, 0, r1E, spE)
                pool_cmax_piece(L, 2)
                mark("G1")
                group_out(E, GROUP, r1E, spE)
                pool_cmax_piece(L, 3)
                pool_cmax_piece(L, 4)
                mark("rowsum_L_h0")
                pe_rowsum_half(L, 0)
                mark("cmaxL_tails")
                dve_cmax_tail(L, 0)
                dve_cmax_tail(L, 1)
                dve_cmax_tail(L, 2)
                mark("G2")
                group_out(E, 2 * GROUP, r1E, spE)
                mark("rowsum_L_h1")
                pe_rowsum_half(L, 64)
                dve_cmax_tail(L, 3)
                dve_cmax_tail(L, 4)
                # L small chains
                mark("sw_L_chain")
                act_s_affine("w", L, ps_cs[:, L, :])
                dve_s_max("w", L, cmax[L])
                pe_conv("w", L)
                wwL, gwL = act_sigmoid_gate("w", L)
                mark("sh_L_chain")
                act_s_affine("h", L, ps_rs[:, L, :])
                dve_s_max("h", L, rmax[L])
                mark("G3")
                group_out(E, 3 * GROUP, r1E, spE)
                pe_conv("h", L)
                whL, ghL = act_sigmoid_gate("h", L)
                mark("lam_L")
                lamw_aL, r1L = dve_lam(L, ghL, gwL, whL)
                spL = dve_spp(L, lamw_aL, wwL, aprime[L])
                mark("G4-7")
                for g in range(4):
                    group_out(L, g * GROUP, r1L, spL)

            for _ in range(reps):
                _rep_body()

    nc.compile()
    return nc


_NC_CACHE = {}


def _get_nc(pr):
    key = tuple(sorted((k, v) for k, v in pr.items()
                       if isinstance(v, float)))
    if key not in _NC_CACHE:
        _NC_CACHE[key] = _build(pr)
    return _NC_CACHE[key]


def kernel(**inputs) -> np.ndarray:
    pr = _fold_params(inputs)
    nc = _get_nc(pr)
    x = np.ascontiguousarray(np.asarray(inputs["x"], dtype=np.float32))
    base = {
        "weff_h": pr["weff_h"], "weff_w": pr["weff_w"],
        "q_h": pr["q_h"], "q_w": pr["q_w"],
        "gate_a": pr["gate_a"], "gate_b": pr["gate_b"],
        "fc1t": pr["fc1t"], "fc2t": pr["fc2t"],
    }
    base = {k: np.ascontiguousarray(v) for k, v in base.items()}
    in_maps = [{**base, "x": x[b]} for b in range(B)]
    res = run_bass_kernel_spmd(nc, in_maps, core_ids=list(range(B)))
    return np.stack([res.results[b]["out"] for b in range(B)], axis=0)
